# revision 15
# baseline (speedup 1.0000x reference)
"""Trainium2 Bass kernel for a continuous-time diagonal SSM layer (S5/S4D-style).

Math (see reference):
    A = exp(Lambda * step)                 (P,) complex, |A| = r, arg = theta
    Bu[t] = B_bar @ u[t]                   (L, P) complex
    x[t]  = A * x[t-1] + Bu[t]             diagonal complex scan
    ys    = 2 Re(C_tilde @ x) + D * u

Kernel strategy (8 cores, sequence-parallel over L):
  * core i owns a contiguous span of LC = L/8 timesteps, chunked by T=512.
  * rotation trick with a SPAN-GLOBAL basis: with A = r e^{i.th}, define
    y[t] = e^{-i.th.t} x[t] (t = span-local index).  y obeys a REAL
    per-partition recurrence y[t] = r y[t-1] + w[t], w[t] = e^{-i.th.t} Bu[t],
    mapping onto the DVE tensor_tensor_scan.  Chunk scans chain by passing
    initial = previous chunk's last output column - no carry-rotation ops.
  * cross-core carry: each core AllGathers its span-final scan state (1KB;
    the AllGather is kicked right after the last scan and its latency hides
    behind phase-2 unrotation), combines with host-precomputed decay weights
    into a per-partition complex scalar Cin; correction y += r^{t+1}*Cin is
    ONE fused scalar_tensor_tensor op per component per chunk.
  * D*u is added on the HOST after gather (saves 8 PE matmuls per chunk).
  * engine balance (cost-model driven): DVE 297ns per bf16 tensor-tensor
    (2x mode), 594ns per scan/stt; Pool 1111ns per op and NO PSUM access;
    Act copies PSUM->SBUF (754-1506ns).  Per chunk p1: DVE 3 mult + 1 add +
    2 scans, Pool 1 mult + 1 sub; p2: DVE 2 stt + 2 mult + 2 add/sub, Pool
    2 mult.  Big DMAs ride SP/Act HWDGE queues (gpsimd-issued DMAs occupy
    the Pool ENGINE for the whole transfer); startup order puts chunk-0
    dependencies (btr/bti blob, ut0, csn0) first.
"""

import numpy as np
import ml_dtypes

import concourse.bass as bass
import concourse.bacc as bacc
import concourse.tile as tile
import concourse.mybir as mybir
from concourse.bass_utils import run_bass_kernel_spmd

F32 = mybir.dt.float32
BF16 = mybir.dt.bfloat16
NPBF16 = ml_dtypes.bfloat16
AX = mybir.AxisListType.X
MUL = mybir.AluOpType.mult
ADD = mybir.AluOpType.add

L, H, P = 32768, 256, 128
NCORES = 8
LC = L // NCORES          # 4096 timesteps per core
T = 512                   # chunk length
NCH = LC // T             # 8 chunks per core

# phase-1 weight blob (bf16): Bbar^T halves
_WB_BTR0 = 0          # Re(Bbar).T rows   0:128  -> [128h, 128p]
_WB_BTR1 = 128
_WB_BTI0 = 256
_WB_BTI1 = 384
_WB_LEN = 512
# phase-2 weight blob (bf16): projection
_WC_CRT = 0           # 2*Re(C).T   [128p, 256h]
_WC_CIT = 256         # -2*Im(C).T
_WC_LEN = 512


def _build_kernel(single=False):
    nc = bacc.Bacc(
        "TRN2", target_bir_lowering=False, debug=False,
        enable_asserts=False, num_devices=1 if single else NCORES,
    )
    # ---------------- per-core external I/O ----------------
    ut_d = nc.dram_tensor("ut", [P, 2, LC], BF16, kind="ExternalInput").ap()
    csn_d = nc.dram_tensor("csn", [P, NCH, 2, T], BF16, kind="ExternalInput").ap()
    rpw_d = nc.dram_tensor("rpw", [P, NCH, T], BF16, kind="ExternalInput").ap()
    wb_d = nc.dram_tensor("wb", [P, _WB_LEN], BF16, kind="ExternalInput").ap()
    wc_d = nc.dram_tensor("wc", [P, _WC_LEN], BF16, kind="ExternalInput").ap()
    fb_d = nc.dram_tensor("fb", [P, T + 2 * NCORES], F32, kind="ExternalInput").ap()
    # natural (p, a, h) order to match the SBUF tile; host re-lays out
    out_d = nc.dram_tensor("out", [NCH, P, 4, H], BF16, kind="ExternalOutput").ap()

    with tile.TileContext(nc) as tc:
        _body(tc, nc, ut_d, csn_d, rpw_d, wb_d, wc_d, fb_d, out_d, single=single)
    nc.compile()
    return nc


def _body(tc, nc, ut_d, csn_d, rpw_d, wb_d, wc_d, fb_d, out_d, single=False):
    with (
        tc.tile_pool(name="const", bufs=1) as cpool,
        tc.tile_pool(name="span", bufs=1) as spool,
        tc.tile_pool(name="work", bufs=3) as wpool,
        tc.tile_pool(name="pbu", bufs=2, space="PSUM") as ppool,
        tc.tile_pool(name="pout", bufs=2, space="PSUM") as opool,
        tc.tile_pool(name="dram", bufs=1, space="DRAM") as dpool,
    ):
        # ---- constants; issue order = dependency order for chunk 0 ----
        wb_sb = cpool.tile([P, _WB_LEN], BF16)
        wc_sb = cpool.tile([P, _WC_LEN], BF16)
        fb_sb = cpool.tile([P, T + 2 * NCORES], F32)
        csn_sb = cpool.tile([P, NCH, 2, T], BF16)
        rpw_sb = cpool.tile([P, NCH, T], BF16)
        nc.sync.dma_start(wb_sb[:], wb_d)
        nc.scalar.dma_start(fb_sb[:], fb_d)
        rrep = fb_sb[:, 0:T]
        wgr = fb_sb[:, T:T + NCORES]
        wgi = fb_sb[:, T + NCORES:T + 2 * NCORES]
        crt = wc_sb[:, _WC_CRT:_WC_CRT + H]
        cit = wc_sb[:, _WC_CIT:_WC_CIT + H]

        # ---- span-persistent state ----
        yr_sb = spool.tile([P, LC], BF16)         # scan outputs (rotated basis)
        yi_sb = spool.tile([P, LC], BF16)

        # =============== phase 1: Bu, rotate, scan ===============
        for c in range(NCH):
            t0 = c * T
            utc = wpool.tile([P, 2, T], BF16, tag="utc")
            nc.sync.dma_start(utc[:], ut_d[:, :, t0:t0 + T])
            nc.scalar.dma_start(csn_sb[:, c], csn_d[:, c])
            # phase-2-only data streams in behind the phase-1 deps
            nc.scalar.dma_start(rpw_sb[:, c], rpw_d[:, c])
            if c == 0:
                nc.sync.dma_start(wc_sb[:], wc_d)
            pbr = ppool.tile([P, T], F32, tag="pbr")
            pbi = ppool.tile([P, T], F32, tag="pbi")
            nc.tensor.matmul(pbr[:], wb_sb[:, _WB_BTR0:_WB_BTR0 + P],
                             utc[:, 0, :], start=True, stop=False)
            nc.tensor.matmul(pbr[:], wb_sb[:, _WB_BTR1:_WB_BTR1 + P],
                             utc[:, 1, :], start=False, stop=True)
            nc.tensor.matmul(pbi[:], wb_sb[:, _WB_BTI0:_WB_BTI0 + P],
                             utc[:, 0, :], start=True, stop=False)
            nc.tensor.matmul(pbi[:], wb_sb[:, _WB_BTI1:_WB_BTI1 + P],
                             utc[:, 1, :], start=False, stop=True)
            cs = csn_sb[:, c, 0, :]
            sn = csn_sb[:, c, 1, :]
            # PSUM -> SBUF bf16 on Act (gpsimd cannot touch PSUM; bf16
            # doubles DVE rate downstream)
            bur = wpool.tile([P, T], BF16, tag="bur")
            bui = wpool.tile([P, T], BF16, tag="bui")
            nc.scalar.copy(bur[:], pbr[:])
            nc.scalar.copy(bui[:], pbi[:])
            # w = e^{-i th t} * Bu
            m1 = wpool.tile([P, T], BF16, tag="m1")
            m2 = wpool.tile([P, T], BF16, tag="m2")
            m3 = wpool.tile([P, T], BF16, tag="m3")
            m4 = wpool.tile([P, T], BF16, tag="m4")
            wr = wpool.tile([P, T], BF16, tag="wr")
            wi = wpool.tile([P, T], BF16, tag="wi")
            nc.gpsimd.tensor_mul(m4[:], sn, bur[:])
            nc.vector.tensor_mul(m1[:], cs, bur[:])
            nc.vector.tensor_mul(m2[:], sn, bui[:])
            nc.vector.tensor_add(wr[:], m1[:], m2[:])
            nc.vector.tensor_mul(m3[:], cs, bui[:])
            nc.gpsimd.tensor_sub(wi[:], m3[:], m4[:])
            # chained scans (span-global basis: init = prev chunk's last col)
            init_r = 0.0 if c == 0 else yr_sb[:, t0 - 1:t0]
            init_i = 0.0 if c == 0 else yi_sb[:, t0 - 1:t0]
            nc.vector.tensor_tensor_scan(
                yr_sb[:, t0:t0 + T], rrep, wr[:], init_r, op0=MUL, op1=ADD)
            nc.vector.tensor_tensor_scan(
                yi_sb[:, t0:t0 + T], rrep, wi[:], init_i, op0=MUL, op1=ADD)

        # =============== cross-core carry: exchange + combine ===============
        e_loc = dpool.tile([P, 2], F32)
        e_all = dpool.tile([NCORES, P, 2], F32)
        e_sb = cpool.tile([P, 2], F32)
        nc.vector.tensor_copy(e_sb[:, 0:1], yr_sb[:, LC - 1:LC])
        nc.vector.tensor_copy(e_sb[:, 1:2], yi_sb[:, LC - 1:LC])
        nc.gpsimd.dma_start(e_loc[:], e_sb[:])
        if single:
            nc.gpsimd.dma_start(e_all[0], e_loc[:])
        else:
            nc.gpsimd.collective_compute(
                "AllGather", mybir.AluOpType.bypass,
                replica_groups=[list(range(NCORES))],
                ins=[e_loc.opt()], outs=[e_all.opt()])
        eall_sb = cpool.tile([P, NCORES, 2], F32)
        nc.sync.dma_start(eall_sb[:],
                          e_all[:, :, :].rearrange("j p c -> p j c"))
        # Cin = sum_j W'_j * E_j   (complex, W' host-precomputed per core)
        er_v = eall_sb[:, :, 0]
        ei_v = eall_sb[:, :, 1]
        ta = cpool.tile([P, NCORES], F32)
        tb = cpool.tile([P, NCORES], F32)
        cr = cpool.tile([P, 1], F32)
        ci = cpool.tile([P, 1], F32)
        nc.vector.tensor_mul(ta[:], wgr, er_v)
        nc.vector.tensor_mul(tb[:], wgi, ei_v)
        nc.vector.tensor_sub(ta[:], ta[:], tb[:])
        nc.vector.tensor_reduce(cr[:], ta[:], axis=AX, op=ADD)
        nc.vector.tensor_mul(ta[:], wgi, er_v)
        nc.vector.tensor_mul(tb[:], wgr, ei_v)
        nc.vector.tensor_add(ta[:], ta[:], tb[:])
        nc.vector.tensor_reduce(ci[:], ta[:], axis=AX, op=ADD)

        # =============== phase 2: correct, unrotate, project ===============
        for c in range(NCH):
            t0 = c * T
            cs = csn_sb[:, c, 0, :]
            sn = csn_sb[:, c, 1, :]
            # y += r^{t+1} * Cin   (one fused op per component)
            yrc = wpool.tile([P, T], BF16, tag="yrc")
            yic = wpool.tile([P, T], BF16, tag="yic")
            nc.vector.scalar_tensor_tensor(
                yrc[:], rpw_sb[:, c], cr[:], yr_sb[:, t0:t0 + T],
                op0=MUL, op1=ADD)
            nc.vector.scalar_tensor_tensor(
                yic[:], rpw_sb[:, c], ci[:], yi_sb[:, t0:t0 + T],
                op0=MUL, op1=ADD)
            # x = e^{+i th t} * y
            n1 = wpool.tile([P, T], BF16, tag="n1")
            n2 = wpool.tile([P, T], BF16, tag="n2")
            n3 = wpool.tile([P, T], BF16, tag="n3")
            n4 = wpool.tile([P, T], BF16, tag="n4")
            xr = wpool.tile([P, T], BF16, tag="xr")
            xi = wpool.tile([P, T], BF16, tag="xi")
            nc.gpsimd.tensor_mul(n2[:], sn, yic[:])
            nc.vector.tensor_mul(n1[:], cs, yrc[:])
            nc.vector.tensor_sub(xr[:], n1[:], n2[:])
            nc.gpsimd.tensor_mul(n4[:], sn, yrc[:])
            nc.vector.tensor_mul(n3[:], cs, yic[:])
            nc.vector.tensor_add(xi[:], n3[:], n4[:])
            # ys[t,h] = 2Re(C x)[t,h]
            po = opool.tile([P, 4, H], F32, tag="po")
            ob = wpool.tile([P, 4, H], BF16, tag="ob")
            for a in range(4):
                sl = slice(a * P, (a + 1) * P)
                nc.tensor.matmul(po[:, a, :], xr[:, sl], crt,
                                 start=True, stop=False)
                nc.tensor.matmul(po[:, a, :], xi[:, sl], cit,
                                 start=False, stop=True)
            nc.scalar.copy(ob[:], po[:])
            nc.sync.dma_start(out_d[c], ob[:])


_NC_CACHE = {}


def _get_nc():
    if "nc" not in _NC_CACHE:
        _NC_CACHE["nc"] = _build_kernel()
    return _NC_CACHE["nc"]


def _host_prep(Lambda_re, Lambda_im, B, C, D, log_step, input_sequence):
    """f64 host-side parameter/table preparation -> per-core input maps."""
    Lam = Lambda_re.astype(np.float64) + 1j * Lambda_im.astype(np.float64)
    step = np.exp(log_step.astype(np.float64))
    A = np.exp(Lam * step)                        # (P,)
    r = np.abs(A)
    th = np.imag(Lam * step)
    Bt = B[..., 0].astype(np.float64) + 1j * B[..., 1].astype(np.float64)
    Bbar = ((A - 1.0) / Lam)[:, None] * Bt        # (P, H)
    Ct = C[..., 0].astype(np.float64) + 1j * C[..., 1].astype(np.float64)  # (H, P)

    s = np.arange(LC, dtype=np.float64)
    ang = th[:, None] * s[None, :]
    cs = np.cos(ang)
    sn = np.sin(ang)
    csn = np.stack([cs.reshape(P, NCH, T), sn.reshape(P, NCH, T)],
                   axis=2).astype(NPBF16)          # [P, NCH, 2, T]
    rpw = np.exp(np.log(r)[:, None] * (s[None, :] + 1.0)) \
        .reshape(P, NCH, T).astype(NPBF16)

    Br = np.real(Bbar).T                          # (256h, 128p)
    Bi = np.imag(Bbar).T
    crt = 2.0 * np.real(Ct).T                     # (128p, 256h)
    cit = -2.0 * np.imag(Ct).T
    wb = np.concatenate([Br[0:P], Br[P:H], Bi[0:P], Bi[P:H]],
                        axis=1).astype(NPBF16)
    wc = np.concatenate([crt, cit], axis=1).astype(NPBF16)

    rrep = np.broadcast_to(r[:, None], (P, T)).astype(np.float32)
    ALC = A ** LC
    eE = np.exp(1j * th * LC)      # includes the carry-in e^{i th} rotation
    # W'[i, j] = ALC^{i-1-j} * eE  for j < i
    wgc = np.zeros((NCORES, P, NCORES), np.complex128)
    pw = np.ones((P,), np.complex128)
    for k in range(NCORES - 1):
        w = pw * eE
        for j in range(NCORES - 1 - k):
            wgc[j + k + 1, :, j] = w
        pw = pw * ALC

    ub = input_sequence.astype(NPBF16)
    uT = ub.T                                     # (256, L) view

    in_maps = []
    for i in range(NCORES):
        utc = np.ascontiguousarray(
            uT[:, i * LC:(i + 1) * LC].reshape(2, P, LC).transpose(1, 0, 2))
        fb = np.concatenate(
            [rrep,
             np.ascontiguousarray(np.real(wgc[i])).astype(np.float32),
             np.ascontiguousarray(np.imag(wgc[i])).astype(np.float32)],
            axis=1)
        in_maps.append({
            "ut": utc,
            "csn": csn,
            "rpw": rpw,
            "wb": wb,
            "wc": wc,
            "fb": fb,
        })
    return in_maps


def kernel(Lambda_re, Lambda_im, B, C, D, log_step, input_sequence):
    in_maps = _host_prep(Lambda_re, Lambda_im, B, C, D, log_step,
                         input_sequence)
    nc = _get_nc()
    res = run_bass_kernel_spmd(nc, in_maps, list(range(NCORES)))
    out = np.concatenate(
        [_unscramble(res.results[i]["out"]) for i in range(NCORES)], axis=0)
    # D*u is cheaper on the host than 8 PE matmuls per chunk on device
    out += D.astype(np.float32) * input_sequence
    return out


def _unscramble(out_arr):
    """device layout [NCH, P, 4, H] (p-major) bf16 -> time-major [LC, H] f32"""
    return (np.asarray(out_arr).astype(np.float32)
            .transpose(0, 2, 1, 3).reshape(LC, H))


if __name__ == "__main__":
    pass


# revision 19
# speedup vs baseline: 1.1372x; 1.1372x over previous
"""Trainium2 Bass kernel for a continuous-time diagonal SSM layer (S5/S4D-style).

Math (see reference):
    A = exp(Lambda * step)                 (P,) complex, |A| = r, arg = theta
    Bu[t] = B_bar @ u[t]                   (L, P) complex
    x[t]  = A * x[t-1] + Bu[t]             diagonal complex scan
    ys    = 2 Re(C_tilde @ x) + D * u

Kernel strategy (8 cores, sequence-parallel over L):
  * core i owns a contiguous span of LC = L/8 timesteps, chunked by T=512.
  * rotation trick with a SPAN-GLOBAL basis: with A = r e^{i.th}, define
    y[t] = e^{-i.th.t} x[t] (t = span-local index).  y obeys a REAL
    per-partition recurrence y[t] = r y[t-1] + w[t], w[t] = e^{-i.th.t} Bu[t],
    mapping onto the DVE tensor_tensor_scan.  Chunk scans chain by passing
    initial = previous chunk's last output column - no carry-rotation ops.
  * cross-core carry: each core AllGathers its span-final scan state (1KB),
    combines with host-precomputed decay weights W'_j = A^{LC(i-1-j)}e^{i.th.LC}
    into a per-partition complex scalar Cin = cr + i*ci.  The correction
    x[t] += e^{i.th.t} r^{t+1} Cin is applied IN OUTPUT SPACE:
    ys_corr[t,h] = sum_p TR[p,t] W1[p,h] + TI[p,t] W2[p,h], with
    TR/TI = r^{t+1} (cos, sin)(th t)  [NOT th(t+1): Cin already carries one
    e^{i.th} from the host weights] and W1 = crt*cr + cit*ci,
    W2 = cit*cr - crt*ci folded once after the collective - two extra PE
    matmuls per output subtile, zero extra DVE passes.
  * D*u is added on the HOST after gather (saves 8 PE matmuls per chunk).
  * schedule: the 3-hop carry chain (sbuf->dram, gather, dram->sbuf) costs
    ~2us per hop in queue/semaphore latency, so phase 2 is split: unrotation
    (collective-independent, DVE) runs first and hides the chain; the Cin
    combine + W1/W2 fold are emitted after 3 unrot chunks; projection
    chunks then interleave with remaining unrot chunks to feed PE early and
    continuously (PE drops out of its slow p-state after 3us of busy).
    Pool (gpsimd) takes 2 of the 6 rotation ops per p1 chunk and the n2/n4
    mults of the LAST 3 unrot chunks (emitted after the collective so the
    in-order gpsimd queue launches the AllGather with no backlog).  Big DMAs
    ride SP/Act HWDGE queues - a gpsimd-issued DMA occupies the Pool ENGINE
    for the whole transfer.
"""

import numpy as np
import ml_dtypes

import concourse.bass as bass
import concourse.bacc as bacc
import concourse.tile as tile
import concourse.mybir as mybir
from concourse.bass_utils import run_bass_kernel_spmd

F32 = mybir.dt.float32
BF16 = mybir.dt.bfloat16
NPBF16 = ml_dtypes.bfloat16
AX = mybir.AxisListType.X
MUL = mybir.AluOpType.mult
ADD = mybir.AluOpType.add

L, H, P = 32768, 256, 128
NCORES = 8
LC = L // NCORES          # 4096 timesteps per core
T = 512                   # chunk length
NCH = LC // T             # 8 chunks per core
POOL_P2A_FROM = 5         # chunks >= this get Pool help in unrotation

# phase-1 weight blob (bf16): Bbar^T halves
_WB_BTR0 = 0          # Re(Bbar).T rows   0:128  -> [128h, 128p]
_WB_BTR1 = 128
_WB_BTI0 = 256
_WB_BTI1 = 384
_WB_LEN = 512
# phase-2 weight blob (bf16): projection
_WC_CRT = 0           # 2*Re(C).T   [128p, 256h]
_WC_CIT = 256         # -2*Im(C).T
_WC_LEN = 512


def _build_kernel(single=False):
    nc = bacc.Bacc(
        "TRN2", target_bir_lowering=False, debug=False,
        enable_asserts=False, num_devices=1 if single else NCORES,
    )
    # ---------------- per-core external I/O ----------------
    ut_d = nc.dram_tensor("ut", [P, 2, LC], BF16, kind="ExternalInput").ap()
    csn_d = nc.dram_tensor("csn", [P, NCH, 2, T], BF16, kind="ExternalInput").ap()
    tt_d = nc.dram_tensor("tt", [P, NCH, 2, T], BF16, kind="ExternalInput").ap()
    wb_d = nc.dram_tensor("wb", [P, _WB_LEN], BF16, kind="ExternalInput").ap()
    wc_d = nc.dram_tensor("wc", [P, _WC_LEN], BF16, kind="ExternalInput").ap()
    fb_d = nc.dram_tensor("fb", [P, T + 2 * NCORES], F32, kind="ExternalInput").ap()
    # natural (p, a, h) order to match the SBUF tile; host re-lays out
    out_d = nc.dram_tensor("out", [NCH, P, 4, H], BF16, kind="ExternalOutput").ap()

    with tile.TileContext(nc) as tc:
        _body(tc, nc, ut_d, csn_d, tt_d, wb_d, wc_d, fb_d, out_d, single=single)
    nc.compile()
    return nc


def _body(tc, nc, ut_d, csn_d, tt_d, wb_d, wc_d, fb_d, out_d, single=False):
    with (
        tc.tile_pool(name="const", bufs=1) as cpool,
        tc.tile_pool(name="span", bufs=1) as spool,
        tc.tile_pool(name="work", bufs=3) as wpool,
        tc.tile_pool(name="pbu", bufs=2, space="PSUM") as ppool,
        tc.tile_pool(name="pout", bufs=2, space="PSUM") as opool,
        tc.tile_pool(name="dram", bufs=1, space="DRAM") as dpool,
    ):
        # ---- constants; issue order = dependency order for chunk 0 ----
        wb_sb = cpool.tile([P, _WB_LEN], BF16)
        wc_sb = cpool.tile([P, _WC_LEN], BF16)
        fb_sb = cpool.tile([P, T + 2 * NCORES], F32)
        csn_sb = cpool.tile([P, NCH, 2, T], BF16)
        tt_sb = cpool.tile([P, NCH, 2, T], BF16)
        nc.sync.dma_start(wb_sb[:], wb_d)
        rrep = fb_sb[:, 0:T]
        wgr = fb_sb[:, T:T + NCORES]
        wgi = fb_sb[:, T + NCORES:T + 2 * NCORES]
        crt = wc_sb[:, _WC_CRT:_WC_CRT + H]
        cit = wc_sb[:, _WC_CIT:_WC_CIT + H]

        # ---- span-persistent state ----
        yr_sb = spool.tile([P, LC], BF16)         # scan outputs (rotated basis)
        yi_sb = spool.tile([P, LC], BF16)
        xr_sb = spool.tile([P, LC], BF16)         # unrotated state
        xi_sb = spool.tile([P, LC], BF16)

        # =============== phase 1: Bu, rotate, scan ===============
        for c in range(NCH):
            t0 = c * T
            utc = wpool.tile([P, 2, T], BF16, tag="utc")
            nc.sync.dma_start(utc[:], ut_d[:, :, t0:t0 + T])
            nc.scalar.dma_start(csn_sb[:, c], csn_d[:, c])
            if c == 0:
                nc.scalar.dma_start(fb_sb[:], fb_d)
            if c == 1:
                nc.sync.dma_start(wc_sb[:], wc_d)
            pbr = ppool.tile([P, T], F32, tag="pbr")
            pbi = ppool.tile([P, T], F32, tag="pbi")
            nc.tensor.matmul(pbr[:], wb_sb[:, _WB_BTR0:_WB_BTR0 + P],
                             utc[:, 0, :], start=True, stop=False)
            nc.tensor.matmul(pbr[:], wb_sb[:, _WB_BTR1:_WB_BTR1 + P],
                             utc[:, 1, :], start=False, stop=True)
            nc.tensor.matmul(pbi[:], wb_sb[:, _WB_BTI0:_WB_BTI0 + P],
                             utc[:, 0, :], start=True, stop=False)
            nc.tensor.matmul(pbi[:], wb_sb[:, _WB_BTI1:_WB_BTI1 + P],
                             utc[:, 1, :], start=False, stop=True)
            cs = csn_sb[:, c, 0, :]
            sn = csn_sb[:, c, 1, :]
            # PSUM -> SBUF bf16 on Act (gpsimd cannot touch PSUM; bf16
            # doubles DVE rate downstream)
            bur = wpool.tile([P, T], BF16, tag="bur")
            bui = wpool.tile([P, T], BF16, tag="bui")
            nc.scalar.copy(bur[:], pbr[:])
            nc.scalar.copy(bui[:], pbi[:])
            # w = e^{-i th t} * Bu
            m1 = wpool.tile([P, T], BF16, tag="m1")
            m2 = wpool.tile([P, T], BF16, tag="m2")
            m3 = wpool.tile([P, T], BF16, tag="m3")
            m4 = wpool.tile([P, T], BF16, tag="m4")
            wr = wpool.tile([P, T], BF16, tag="wr")
            wi = wpool.tile([P, T], BF16, tag="wi")
            nc.vector.tensor_mul(m1[:], cs, bur[:])
            nc.gpsimd.tensor_mul(m4[:], sn, bur[:])
            nc.vector.tensor_mul(m2[:], sn, bui[:])
            nc.vector.tensor_add(wr[:], m1[:], m2[:])
            nc.vector.tensor_mul(m3[:], cs, bui[:])
            nc.gpsimd.tensor_sub(wi[:], m3[:], m4[:])
            # phase-2-only table streams in behind the phase-1 deps
            nc.scalar.dma_start(tt_sb[:, c], tt_d[:, c])
            # chained scans (span-global basis: init = prev chunk's last col)
            init_r = 0.0 if c == 0 else yr_sb[:, t0 - 1:t0]
            init_i = 0.0 if c == 0 else yi_sb[:, t0 - 1:t0]
            nc.vector.tensor_tensor_scan(
                yr_sb[:, t0:t0 + T], rrep, wr[:], init_r, op0=MUL, op1=ADD)
            nc.vector.tensor_tensor_scan(
                yi_sb[:, t0:t0 + T], rrep, wi[:], init_i, op0=MUL, op1=ADD)

        # =============== kick off cross-core carry exchange ===============
        e_loc = dpool.tile([P, 2], F32)
        e_all = dpool.tile([NCORES, P, 2], F32)
        e_sb = cpool.tile([P, 2], F32)
        nc.vector.tensor_copy(e_sb[:, 0:1], yr_sb[:, LC - 1:LC])
        nc.vector.tensor_copy(e_sb[:, 1:2], yi_sb[:, LC - 1:LC])
        nc.sync.dma_start(e_loc[:], e_sb[:])
        if single:
            nc.scalar.dma_start(e_all[0], e_loc[:])
        else:
            nc.gpsimd.collective_compute(
                "AllGather", mybir.AluOpType.bypass,
                replica_groups=[list(range(NCORES))],
                ins=[e_loc.opt()], outs=[e_all.opt()])

        def p2a(c):
            """unrotate x = e^{+i th t} * y (collective-independent)"""
            t0 = c * T
            cs = csn_sb[:, c, 0, :]
            sn = csn_sb[:, c, 1, :]
            n1 = wpool.tile([P, T], BF16, tag="n1")
            n2 = wpool.tile([P, T], BF16, tag="n2")
            n3 = wpool.tile([P, T], BF16, tag="n3")
            n4 = wpool.tile([P, T], BF16, tag="n4")
            eng = nc.gpsimd if c >= POOL_P2A_FROM else nc.vector
            eng.tensor_mul(n2[:], sn, yi_sb[:, t0:t0 + T])
            nc.vector.tensor_mul(n1[:], cs, yr_sb[:, t0:t0 + T])
            nc.vector.tensor_sub(xr_sb[:, t0:t0 + T], n1[:], n2[:])
            eng.tensor_mul(n4[:], sn, yr_sb[:, t0:t0 + T])
            nc.vector.tensor_mul(n3[:], cs, yi_sb[:, t0:t0 + T])
            nc.vector.tensor_add(xi_sb[:, t0:t0 + T], n3[:], n4[:])

        def p2b(c):
            """project ys[t,h] = 2Re(C x)[t,h] + carry (TR W1 + TI W2)"""
            t0 = c * T
            po = opool.tile([P, 4, H], F32, tag="po")
            ob = wpool.tile([P, 4, H], BF16, tag="ob")
            for a in range(4):
                sl = slice(t0 + a * P, t0 + (a + 1) * P)
                nc.tensor.matmul(po[:, a, :], xr_sb[:, sl], crt,
                                 start=True, stop=False)
                nc.tensor.matmul(po[:, a, :], xi_sb[:, sl], cit,
                                 start=False, stop=False)
                nc.tensor.matmul(po[:, a, :], tt_sb[:, c, 0, a * P:(a + 1) * P],
                                 w1[:], start=False, stop=False)
                nc.tensor.matmul(po[:, a, :], tt_sb[:, c, 1, a * P:(a + 1) * P],
                                 w2[:], start=False, stop=True)
            nc.scalar.copy(ob[:], po[:])
            nc.sync.dma_start(out_d[c], ob[:])

        # unrot leads; the carry chain's ~3 hops hide behind it
        p2a(0)
        p2a(1)
        p2a(2)

        # =============== carry combine -> W1/W2 ===============
        eall_sb = cpool.tile([P, NCORES, 2], F32)
        nc.sync.dma_start(eall_sb[:],
                          e_all[:, :, :].rearrange("j p c -> p j c"))
        er_v = eall_sb[:, :, 0]
        ei_v = eall_sb[:, :, 1]
        ta = cpool.tile([P, NCORES], F32)
        tb = cpool.tile([P, NCORES], F32)
        cr = cpool.tile([P, 1], F32)
        ci = cpool.tile([P, 1], F32)
        nc.vector.tensor_mul(ta[:], wgr, er_v)
        nc.vector.tensor_mul(tb[:], wgi, ei_v)
        nc.vector.tensor_sub(ta[:], ta[:], tb[:])
        nc.vector.tensor_reduce(cr[:], ta[:], axis=AX, op=ADD)
        nc.vector.tensor_mul(ta[:], wgi, er_v)
        nc.vector.tensor_mul(tb[:], wgr, ei_v)
        nc.vector.tensor_add(ta[:], ta[:], tb[:])
        nc.vector.tensor_reduce(ci[:], ta[:], axis=AX, op=ADD)
        w1 = cpool.tile([P, H], BF16)
        w2 = cpool.tile([P, H], BF16)
        tw1 = cpool.tile([P, H], BF16)
        tw2 = cpool.tile([P, H], BF16)
        nc.vector.tensor_scalar_mul(tw1[:], crt, cr[:])
        nc.vector.tensor_scalar_mul(tw2[:], cit, ci[:])
        nc.vector.tensor_add(w1[:], tw1[:], tw2[:])
        nc.vector.tensor_scalar_mul(tw1[:], cit, cr[:])
        nc.vector.tensor_scalar_mul(tw2[:], crt, ci[:])
        nc.vector.tensor_sub(w2[:], tw1[:], tw2[:])

        # =============== interleaved projection / remaining unrot ==========
        p2b(0)
        for c in range(3, NCH):
            p2a(c)
            p2b(c - 2)
        p2b(NCH - 2)
        p2b(NCH - 1)


_NC_CACHE = {}


def _get_nc():
    if "nc" not in _NC_CACHE:
        _NC_CACHE["nc"] = _build_kernel()
    return _NC_CACHE["nc"]


def _host_prep(Lambda_re, Lambda_im, B, C, D, log_step, input_sequence):
    """f64 host-side parameter/table preparation -> per-core input maps."""
    Lam = Lambda_re.astype(np.float64) + 1j * Lambda_im.astype(np.float64)
    step = np.exp(log_step.astype(np.float64))
    A = np.exp(Lam * step)                        # (P,)
    r = np.abs(A)
    th = np.imag(Lam * step)
    Bt = B[..., 0].astype(np.float64) + 1j * B[..., 1].astype(np.float64)
    Bbar = ((A - 1.0) / Lam)[:, None] * Bt        # (P, H)
    Ct = C[..., 0].astype(np.float64) + 1j * C[..., 1].astype(np.float64)  # (H, P)

    s = np.arange(LC, dtype=np.float64)
    ang = th[:, None] * s[None, :]
    cs = np.cos(ang)
    sn = np.sin(ang)
    csn = np.stack([cs.reshape(P, NCH, T), sn.reshape(P, NCH, T)],
                   axis=2).astype(NPBF16)          # [P, NCH, 2, T]
    # TR/TI = r^{t+1} (cos, sin)(th t): the e^{i th (t+1)} of A^{t+1} is
    # split as e^{i th t} here x e^{i th} inside the host carry weights
    rp = np.exp(np.log(r)[:, None] * (s[None, :] + 1.0))
    tr = rp * cs
    ti = rp * sn
    tt = np.stack([tr.reshape(P, NCH, T), ti.reshape(P, NCH, T)],
                  axis=2).astype(NPBF16)           # [P, NCH, 2, T]

    Br = np.real(Bbar).T                          # (256h, 128p)
    Bi = np.imag(Bbar).T
    crt = 2.0 * np.real(Ct).T                     # (128p, 256h)
    cit = -2.0 * np.imag(Ct).T
    wb = np.concatenate([Br[0:P], Br[P:H], Bi[0:P], Bi[P:H]],
                        axis=1).astype(NPBF16)
    wc = np.concatenate([crt, cit], axis=1).astype(NPBF16)

    rrep = np.broadcast_to(r[:, None], (P, T)).astype(np.float32)
    ALC = A ** LC
    eE = np.exp(1j * th * LC)      # includes the carry-in e^{i th} rotation
    # W'[i, j] = ALC^{i-1-j} * eE  for j < i
    wgc = np.zeros((NCORES, P, NCORES), np.complex128)
    pw = np.ones((P,), np.complex128)
    for k in range(NCORES - 1):
        w = pw * eE
        for j in range(NCORES - 1 - k):
            wgc[j + k + 1, :, j] = w
        pw = pw * ALC

    ub = input_sequence.astype(NPBF16)
    uT = ub.T                                     # (256, L) view

    in_maps = []
    for i in range(NCORES):
        utc = np.ascontiguousarray(
            uT[:, i * LC:(i + 1) * LC].reshape(2, P, LC).transpose(1, 0, 2))
        fb = np.concatenate(
            [rrep,
             np.ascontiguousarray(np.real(wgc[i])).astype(np.float32),
             np.ascontiguousarray(np.imag(wgc[i])).astype(np.float32)],
            axis=1)
        in_maps.append({
            "ut": utc,
            "csn": csn,
            "tt": tt,
            "wb": wb,
            "wc": wc,
            "fb": fb,
        })
    return in_maps


def kernel(Lambda_re, Lambda_im, B, C, D, log_step, input_sequence):
    in_maps = _host_prep(Lambda_re, Lambda_im, B, C, D, log_step,
                         input_sequence)
    nc = _get_nc()
    res = run_bass_kernel_spmd(nc, in_maps, list(range(NCORES)))
    out = np.concatenate(
        [_unscramble(res.results[i]["out"]) for i in range(NCORES)], axis=0)
    # D*u is cheaper on the host than 8 PE matmuls per chunk on device
    out += D.astype(np.float32) * input_sequence
    return out


def _unscramble(out_arr):
    """device layout [NCH, P, 4, H] (p-major) bf16 -> time-major [LC, H] f32"""
    return (np.asarray(out_arr).astype(np.float32)
            .transpose(0, 2, 1, 3).reshape(LC, H))


if __name__ == "__main__":
    pass


# revision 25
# speedup vs baseline: 1.1575x; 1.0179x over previous
"""Trainium2 Bass kernel for a continuous-time diagonal SSM layer (S5/S4D-style).

Math (see reference):
    A = exp(Lambda * step)                 (P,) complex, |A| = r, arg = theta
    Bu[t] = B_bar @ u[t]                   (L, P) complex
    x[t]  = A * x[t-1] + Bu[t]             diagonal complex scan
    ys    = 2 Re(C_tilde @ x) + D * u

Kernel strategy (8 cores, sequence-parallel over L):
  * core i owns a contiguous span of LC = L/8 timesteps, chunked by T=512.
  * rotation trick with a SPAN-GLOBAL basis: with A = r e^{i.th}, define
    y[t] = e^{-i.th.t} x[t] (t = span-local index).  y obeys a REAL
    per-partition recurrence y[t] = r y[t-1] + w[t], w[t] = e^{-i.th.t} Bu[t],
    mapping onto the DVE tensor_tensor_scan.  Chunk scans chain by passing
    initial = previous chunk's last output column - no carry-rotation ops.
  * cross-core carry: each core AllGathers its span-final scan state (1KB),
    combines with host-precomputed decay weights W'_j = A^{LC(i-1-j)}e^{i.th.LC}
    into a per-partition complex scalar Cin = cr + i*ci.  The correction
    x[t] += e^{i.th.t} r^{t+1} Cin is applied IN OUTPUT SPACE:
    ys_corr[t,h] = sum_p TR[p,t] W1[p,h] + TI[p,t] W2[p,h], with
    TR/TI = r^{t+1} (cos, sin)(th t)  [NOT th(t+1): Cin already carries one
    e^{i.th} from the host weights] and W1 = crt*cr + cit*ci,
    W2 = cit*cr - crt*ci folded once after the collective - two extra PE
    matmuls per output subtile, zero extra DVE passes.
  * D*u is added on the HOST after gather (saves 8 PE matmuls per chunk).
  * schedule: the 3-hop carry chain (sbuf->dram, gather, dram->sbuf) costs
    ~2us per hop in queue/semaphore latency, so phase 2 is split: unrotation
    (collective-independent, DVE) runs first and hides the chain; the Cin
    combine + W1/W2 fold are emitted after 3 unrot chunks; projection
    chunks then interleave with remaining unrot chunks to feed PE early and
    continuously (PE drops out of its slow p-state after 3us of busy).
    Pool (gpsimd) takes 2 of the 6 rotation ops per p1 chunk and the n2/n4
    mults of the LAST 3 unrot chunks (emitted after the collective so the
    in-order gpsimd queue launches the AllGather with no backlog).  Big DMAs
    ride SP/Act HWDGE queues - a gpsimd-issued DMA occupies the Pool ENGINE
    for the whole transfer.
"""

import numpy as np
import ml_dtypes

import concourse.bass as bass
import concourse.bacc as bacc
import concourse.tile as tile
import concourse.mybir as mybir
from concourse.bass_utils import run_bass_kernel_spmd

F32 = mybir.dt.float32
BF16 = mybir.dt.bfloat16
NPBF16 = ml_dtypes.bfloat16
AX = mybir.AxisListType.X
MUL = mybir.AluOpType.mult
ADD = mybir.AluOpType.add

L, H, P = 32768, 256, 128
NCORES = 8
LC = L // NCORES          # 4096 timesteps per core
T = 512                   # chunk length
NCH = LC // T             # 8 chunks per core
POOL_P2A_FROM = 4         # chunks >= this get Pool help in unrotation

# phase-1 weight blob (bf16): Bbar^T halves
_WB_BTR0 = 0          # Re(Bbar).T rows   0:128  -> [128h, 128p]
_WB_BTR1 = 128
_WB_BTI0 = 256
_WB_BTI1 = 384
_WB_LEN = 512
# phase-2 weight blob (bf16): projection
_WC_CRT = 0           # 2*Re(C).T   [128p, 256h]
_WC_CIT = 256         # -2*Im(C).T
_WC_LEN = 512


def _build_kernel(single=False):
    nc = bacc.Bacc(
        "TRN2", target_bir_lowering=False, debug=False,
        enable_asserts=False, num_devices=1 if single else NCORES,
    )
    # ---------------- per-core external I/O ----------------
    ut_d = nc.dram_tensor("ut", [P, 2, LC], BF16, kind="ExternalInput").ap()
    csn_d = nc.dram_tensor("csn", [P, NCH, 2, T], BF16, kind="ExternalInput").ap()
    tt_d = nc.dram_tensor("tt", [P, NCH, 2, T], BF16, kind="ExternalInput").ap()
    wb_d = nc.dram_tensor("wb", [P, _WB_LEN], BF16, kind="ExternalInput").ap()
    wc_d = nc.dram_tensor("wc", [P, _WC_LEN], BF16, kind="ExternalInput").ap()
    fb_d = nc.dram_tensor("fb", [P, T + 2 * NCORES], F32, kind="ExternalInput").ap()
    # natural (p, a, h) order to match the SBUF tile; host re-lays out
    out_d = nc.dram_tensor("out", [NCH, P, 4, H], BF16, kind="ExternalOutput").ap()

    with tile.TileContext(nc) as tc:
        _body(tc, nc, ut_d, csn_d, tt_d, wb_d, wc_d, fb_d, out_d, single=single)
    nc.compile()
    return nc


def _body(tc, nc, ut_d, csn_d, tt_d, wb_d, wc_d, fb_d, out_d, single=False):
    with (
        tc.tile_pool(name="const", bufs=1) as cpool,
        tc.tile_pool(name="span", bufs=1) as spool,
        tc.tile_pool(name="work", bufs=3) as wpool,
        tc.tile_pool(name="pbu", bufs=1, space="PSUM") as ppool,
        tc.tile_pool(name="pout", bufs=3, space="PSUM") as opool,
        tc.tile_pool(name="dram", bufs=1, space="DRAM") as dpool,
    ):
        # ---- constants; issue order = dependency order for chunk 0 ----
        wb_sb = cpool.tile([P, _WB_LEN], BF16)
        wc_sb = cpool.tile([P, _WC_LEN], BF16)
        fb_sb = cpool.tile([P, T + 2 * NCORES], F32)
        csn_sb = cpool.tile([P, NCH, 2, T], BF16)
        tt_sb = cpool.tile([P, NCH, 2, T], BF16)
        nc.sync.dma_start(wb_sb[:], wb_d)
        rrep = fb_sb[:, 0:T]
        wgr = fb_sb[:, T:T + NCORES]
        wgi = fb_sb[:, T + NCORES:T + 2 * NCORES]
        crt = wc_sb[:, _WC_CRT:_WC_CRT + H]
        cit = wc_sb[:, _WC_CIT:_WC_CIT + H]

        # ---- span-persistent state ----
        yr_sb = spool.tile([P, LC], BF16)         # scan outputs (rotated basis)
        yi_sb = spool.tile([P, LC], BF16)
        xr_sb = spool.tile([P, LC], BF16)         # unrotated state
        xi_sb = spool.tile([P, LC], BF16)

        # =============== phase 1: Bu, rotate, scan ===============
        for c in range(NCH):
            t0 = c * T
            utc = wpool.tile([P, 2, T], BF16, tag="utc")
            nc.sync.dma_start(utc[:], ut_d[:, :, t0:t0 + T])
            nc.scalar.dma_start(csn_sb[:, c], csn_d[:, c])
            if c == 0:
                nc.scalar.dma_start(fb_sb[:], fb_d)
            if c == 1:
                nc.sync.dma_start(wc_sb[:], wc_d)
            pbr = ppool.tile([P, T], F32, tag="pbr")
            pbi = ppool.tile([P, T], F32, tag="pbi")
            nc.tensor.matmul(pbr[:], wb_sb[:, _WB_BTR0:_WB_BTR0 + P],
                             utc[:, 0, :], start=True, stop=False)
            nc.tensor.matmul(pbr[:], wb_sb[:, _WB_BTR1:_WB_BTR1 + P],
                             utc[:, 1, :], start=False, stop=True)
            nc.tensor.matmul(pbi[:], wb_sb[:, _WB_BTI0:_WB_BTI0 + P],
                             utc[:, 0, :], start=True, stop=False)
            nc.tensor.matmul(pbi[:], wb_sb[:, _WB_BTI1:_WB_BTI1 + P],
                             utc[:, 1, :], start=False, stop=True)
            cs = csn_sb[:, c, 0, :]
            sn = csn_sb[:, c, 1, :]
            # PSUM -> SBUF bf16 on Act (gpsimd cannot touch PSUM; bf16
            # doubles DVE rate downstream)
            bur = wpool.tile([P, T], BF16, tag="bur")
            bui = wpool.tile([P, T], BF16, tag="bui")
            nc.scalar.copy(bur[:], pbr[:])
            nc.scalar.copy(bui[:], pbi[:])
            # w = e^{-i th t} * Bu
            m1 = wpool.tile([P, T], BF16, tag="m1")
            m2 = wpool.tile([P, T], BF16, tag="m2")
            m3 = wpool.tile([P, T], BF16, tag="m3")
            m4 = wpool.tile([P, T], BF16, tag="m4")
            wr = wpool.tile([P, T], BF16, tag="wr")
            wi = wpool.tile([P, T], BF16, tag="wi")
            nc.vector.tensor_mul(m1[:], cs, bur[:])
            nc.gpsimd.tensor_mul(m4[:], sn, bur[:])
            nc.vector.tensor_mul(m2[:], sn, bui[:])
            nc.vector.tensor_add(wr[:], m1[:], m2[:])
            nc.vector.tensor_mul(m3[:], cs, bui[:])
            nc.gpsimd.tensor_sub(wi[:], m3[:], m4[:])
            # phase-2-only table streams in behind the phase-1 deps
            nc.scalar.dma_start(tt_sb[:, c], tt_d[:, c])
            # chained scans (span-global basis: init = prev chunk's last col)
            init_r = 0.0 if c == 0 else yr_sb[:, t0 - 1:t0]
            init_i = 0.0 if c == 0 else yi_sb[:, t0 - 1:t0]
            nc.vector.tensor_tensor_scan(
                yr_sb[:, t0:t0 + T], rrep, wr[:], init_r, op0=MUL, op1=ADD)
            nc.vector.tensor_tensor_scan(
                yi_sb[:, t0:t0 + T], rrep, wi[:], init_i, op0=MUL, op1=ADD)

        # =============== kick off cross-core carry exchange ===============
        e_loc = dpool.tile([P, 2], F32)
        e_all = dpool.tile([NCORES, P, 2], F32)
        e_sb = cpool.tile([P, 2], F32)
        with tc.high_priority():
            nc.vector.tensor_copy(e_sb[:, 0:1], yr_sb[:, LC - 1:LC])
            nc.vector.tensor_copy(e_sb[:, 1:2], yi_sb[:, LC - 1:LC])
            nc.sync.dma_start(e_loc[:], e_sb[:])
            if single:
                nc.scalar.dma_start(e_all[0], e_loc[:])
            else:
                nc.gpsimd.collective_compute(
                    "AllGather", mybir.AluOpType.bypass,
                    replica_groups=[list(range(NCORES))],
                    ins=[e_loc.opt()], outs=[e_all.opt()])

        def p2a(c):
            """unrotate x = e^{+i th t} * y (collective-independent)"""
            t0 = c * T
            cs = csn_sb[:, c, 0, :]
            sn = csn_sb[:, c, 1, :]
            n1 = wpool.tile([P, T], BF16, tag="n1")
            n2 = wpool.tile([P, T], BF16, tag="n2")
            n3 = wpool.tile([P, T], BF16, tag="n3")
            n4 = wpool.tile([P, T], BF16, tag="n4")
            eng = nc.gpsimd if c >= POOL_P2A_FROM else nc.vector
            eng.tensor_mul(n2[:], sn, yi_sb[:, t0:t0 + T])
            nc.vector.tensor_mul(n1[:], cs, yr_sb[:, t0:t0 + T])
            nc.vector.tensor_sub(xr_sb[:, t0:t0 + T], n1[:], n2[:])
            eng.tensor_mul(n4[:], sn, yr_sb[:, t0:t0 + T])
            nc.vector.tensor_mul(n3[:], cs, yi_sb[:, t0:t0 + T])
            nc.vector.tensor_add(xi_sb[:, t0:t0 + T], n3[:], n4[:])

        def p2b(c):
            """project ys[t,h] = 2Re(C x)[t,h] + carry (TR W1 + TI W2)"""
            t0 = c * T
            po = opool.tile([P, 4, H], F32, tag="po")
            ob = wpool.tile([P, 4, H], BF16, tag="ob")
            for a in range(4):
                sl = slice(t0 + a * P, t0 + (a + 1) * P)
                nc.tensor.matmul(po[:, a, :], xr_sb[:, sl], crt,
                                 start=True, stop=False)
                nc.tensor.matmul(po[:, a, :], xi_sb[:, sl], cit,
                                 start=False, stop=False)
                nc.tensor.matmul(po[:, a, :], tt_sb[:, c, 0, a * P:(a + 1) * P],
                                 w1[:], start=False, stop=False)
                nc.tensor.matmul(po[:, a, :], tt_sb[:, c, 1, a * P:(a + 1) * P],
                                 w2[:], start=False, stop=True)
            # late chunks: DVE is drained of unrot work - split the
            # PSUM->SBUF copy with Act to halve tail pacing
            if c >= NCH - 3:
                nc.scalar.copy(ob[:, 0:2, :], po[:, 0:2, :])
                nc.vector.tensor_copy(ob[:, 2:4, :], po[:, 2:4, :])
            else:
                nc.scalar.copy(ob[:], po[:])
            nc.sync.dma_start(out_d[c], ob[:])

        # unrot leads; the carry chain's ~3 hops hide behind it
        p2a(0)
        p2a(1)
        p2a(2)

        # =============== carry combine -> W1/W2 ===============
        # high_priority: W1/W2 gate ~16us of PE projection work; don't let
        # the tile scheduler push these behind the remaining unrot chunks
        eall_sb = cpool.tile([P, NCORES, 2], F32)
        ta = cpool.tile([P, NCORES], F32)
        tb = cpool.tile([P, NCORES], F32)
        cr = cpool.tile([P, 1], F32)
        ci = cpool.tile([P, 1], F32)
        w1 = cpool.tile([P, H], BF16)
        w2 = cpool.tile([P, H], BF16)
        tw1 = cpool.tile([P, H], BF16)
        tw2 = cpool.tile([P, H], BF16)
        with tc.high_priority():
            nc.sync.dma_start(eall_sb[:],
                              e_all[:, :, :].rearrange("j p c -> p j c"))
            er_v = eall_sb[:, :, 0]
            ei_v = eall_sb[:, :, 1]
            nc.vector.tensor_mul(ta[:], wgr, er_v)
            nc.vector.tensor_mul(tb[:], wgi, ei_v)
            nc.vector.tensor_sub(ta[:], ta[:], tb[:])
            nc.vector.tensor_reduce(cr[:], ta[:], axis=AX, op=ADD)
            nc.vector.tensor_mul(ta[:], wgi, er_v)
            nc.vector.tensor_mul(tb[:], wgr, ei_v)
            nc.vector.tensor_add(ta[:], ta[:], tb[:])
            nc.vector.tensor_reduce(ci[:], ta[:], axis=AX, op=ADD)
            nc.vector.tensor_scalar_mul(tw1[:], cit, ci[:])
            nc.vector.scalar_tensor_tensor(w1[:], crt, cr[:], tw1[:],
                                           op0=MUL, op1=ADD)
            nc.vector.tensor_scalar_mul(tw2[:], crt, ci[:])
            nc.vector.scalar_tensor_tensor(w2[:], cit, cr[:], tw2[:],
                                           op0=MUL, op1=mybir.AluOpType.subtract)

        # =============== interleaved projection / remaining unrot ==========
        p2b(0)
        for c in range(3, NCH):
            p2a(c)
            p2b(c - 2)
        p2b(NCH - 2)
        p2b(NCH - 1)


_NC_CACHE = {}


def _get_nc():
    if "nc" not in _NC_CACHE:
        _NC_CACHE["nc"] = _build_kernel()
    return _NC_CACHE["nc"]


def _host_prep(Lambda_re, Lambda_im, B, C, D, log_step, input_sequence):
    """f64 host-side parameter/table preparation -> per-core input maps."""
    Lam = Lambda_re.astype(np.float64) + 1j * Lambda_im.astype(np.float64)
    step = np.exp(log_step.astype(np.float64))
    A = np.exp(Lam * step)                        # (P,)
    r = np.abs(A)
    th = np.imag(Lam * step)
    Bt = B[..., 0].astype(np.float64) + 1j * B[..., 1].astype(np.float64)
    Bbar = ((A - 1.0) / Lam)[:, None] * Bt        # (P, H)
    Ct = C[..., 0].astype(np.float64) + 1j * C[..., 1].astype(np.float64)  # (H, P)

    s = np.arange(LC, dtype=np.float64)
    ang = th[:, None] * s[None, :]
    cs = np.cos(ang)
    sn = np.sin(ang)
    csn = np.stack([cs.reshape(P, NCH, T), sn.reshape(P, NCH, T)],
                   axis=2).astype(NPBF16)          # [P, NCH, 2, T]
    # TR/TI = r^{t+1} (cos, sin)(th t): the e^{i th (t+1)} of A^{t+1} is
    # split as e^{i th t} here x e^{i th} inside the host carry weights
    rp = np.exp(np.log(r)[:, None] * (s[None, :] + 1.0))
    tr = rp * cs
    ti = rp * sn
    tt = np.stack([tr.reshape(P, NCH, T), ti.reshape(P, NCH, T)],
                  axis=2).astype(NPBF16)           # [P, NCH, 2, T]

    Br = np.real(Bbar).T                          # (256h, 128p)
    Bi = np.imag(Bbar).T
    crt = 2.0 * np.real(Ct).T                     # (128p, 256h)
    cit = -2.0 * np.imag(Ct).T
    wb = np.concatenate([Br[0:P], Br[P:H], Bi[0:P], Bi[P:H]],
                        axis=1).astype(NPBF16)
    wc = np.concatenate([crt, cit], axis=1).astype(NPBF16)

    rrep = np.broadcast_to(r[:, None], (P, T)).astype(np.float32)
    ALC = A ** LC
    eE = np.exp(1j * th * LC)      # includes the carry-in e^{i th} rotation
    # W'[i, j] = ALC^{i-1-j} * eE  for j < i
    wgc = np.zeros((NCORES, P, NCORES), np.complex128)
    pw = np.ones((P,), np.complex128)
    for k in range(NCORES - 1):
        w = pw * eE
        for j in range(NCORES - 1 - k):
            wgc[j + k + 1, :, j] = w
        pw = pw * ALC

    ub = input_sequence.astype(NPBF16)
    uT = ub.T                                     # (256, L) view

    in_maps = []
    for i in range(NCORES):
        utc = np.ascontiguousarray(
            uT[:, i * LC:(i + 1) * LC].reshape(2, P, LC).transpose(1, 0, 2))
        fb = np.concatenate(
            [rrep,
             np.ascontiguousarray(np.real(wgc[i])).astype(np.float32),
             np.ascontiguousarray(np.imag(wgc[i])).astype(np.float32)],
            axis=1)
        in_maps.append({
            "ut": utc,
            "csn": csn,
            "tt": tt,
            "wb": wb,
            "wc": wc,
            "fb": fb,
        })
    return in_maps


def kernel(Lambda_re, Lambda_im, B, C, D, log_step, input_sequence):
    in_maps = _host_prep(Lambda_re, Lambda_im, B, C, D, log_step,
                         input_sequence)
    nc = _get_nc()
    res = run_bass_kernel_spmd(nc, in_maps, list(range(NCORES)))
    out = np.concatenate(
        [_unscramble(res.results[i]["out"]) for i in range(NCORES)], axis=0)
    # D*u is cheaper on the host than 8 PE matmuls per chunk on device
    out += D.astype(np.float32) * input_sequence
    return out


def _unscramble(out_arr):
    """device layout [NCH, P, 4, H] (p-major) bf16 -> time-major [LC, H] f32"""
    return (np.asarray(out_arr).astype(np.float32)
            .transpose(0, 2, 1, 3).reshape(LC, H))


if __name__ == "__main__":
    pass


# revision 32
# speedup vs baseline: 1.2125x; 1.0475x over previous
"""Trainium2 Bass kernel for a continuous-time diagonal SSM layer (S5/S4D-style).

Math (see reference):
    A = exp(Lambda * step)                 (P,) complex, |A| = r, arg = theta
    Bu[t] = B_bar @ u[t]                   (L, P) complex
    x[t]  = A * x[t-1] + Bu[t]             diagonal complex scan
    ys    = 2 Re(C_tilde @ x) + D * u

Kernel strategy (8 cores, sequence-parallel over L):
  * core i owns a contiguous span of LC = L/8 timesteps, chunked by T=512.
  * rotation trick with a SPAN-GLOBAL basis: with A = r e^{i.th}, define
    y[t] = e^{-i.th.t} x[t] (t = span-local index).  y obeys a REAL
    per-partition recurrence y[t] = r y[t-1] + w[t], w[t] = e^{-i.th.t} Bu[t],
    mapping onto the DVE tensor_tensor_scan.  Chunk scans chain by passing
    initial = previous chunk's last output column - no carry-rotation ops.
  * cross-core carry: each core AllGathers its span-final scan state (1KB),
    combines with host-precomputed decay weights W'_j = A^{LC(i-1-j)}e^{i.th.LC}
    into a per-partition complex scalar Cin = cr + i*ci.  The correction
    x[t] += e^{i.th.t} r^{t+1} Cin is applied IN OUTPUT SPACE:
    ys_corr[t,h] = sum_p TR[p,t] W1[p,h] + TI[p,t] W2[p,h], with
    TR/TI = r^{t+1} (cos, sin)(th t)  [NOT th(t+1): Cin already carries one
    e^{i.th} from the host weights] and W1 = crt*cr + cit*ci,
    W2 = cit*cr - crt*ci folded once after the collective - two extra PE
    matmuls per output subtile, zero extra DVE passes.
  * D*u is added on the HOST after gather (saves 8 PE matmuls per chunk).
  * schedule: the 3-hop carry chain (sbuf->dram, gather, dram->sbuf) costs
    ~2us per hop in queue/semaphore latency, so phase 2 is split: unrotation
    (collective-independent, DVE) runs first and hides the chain; the Cin
    combine + W1/W2 fold are emitted after 3 unrot chunks; projection
    chunks then interleave with remaining unrot chunks to feed PE early and
    continuously (PE drops out of its slow p-state after 3us of busy).
    Pool (gpsimd) takes 2 of the 6 rotation ops per p1 chunk and the n2/n4
    mults of the LAST 3 unrot chunks (emitted after the collective so the
    in-order gpsimd queue launches the AllGather with no backlog).  Big DMAs
    ride SP/Act HWDGE queues - a gpsimd-issued DMA occupies the Pool ENGINE
    for the whole transfer.
"""

import numpy as np
import ml_dtypes

import concourse.bass as bass
import concourse.bacc as bacc
import concourse.tile as tile
import concourse.mybir as mybir
from concourse.bass_utils import run_bass_kernel_spmd

F32 = mybir.dt.float32
BF16 = mybir.dt.bfloat16
NPBF16 = ml_dtypes.bfloat16
AX = mybir.AxisListType.X
MUL = mybir.AluOpType.mult
ADD = mybir.AluOpType.add

L, H, P = 32768, 256, 128
NCORES = 8
LC = L // NCORES          # 4096 timesteps per core
T = 512                   # chunk length
NCH = LC // T             # 8 chunks per core
POOL_P2A_FROM = 4         # chunks >= this get Pool help in unrotation

# phase-1 weight blob (bf16): Bbar^T halves
_WB_BTR0 = 0          # Re(Bbar).T rows   0:128  -> [128h, 128p]
_WB_BTR1 = 128
_WB_BTI0 = 256
_WB_BTI1 = 384
_WB_LEN = 512
# phase-2 weight blob (bf16): projection
_WC_CRT = 0           # 2*Re(C).T   [128p, 256h]
_WC_CIT = 256         # -2*Im(C).T
_WC_LEN = 512


def _build_kernel(single=False):
    nc = bacc.Bacc(
        "TRN2", target_bir_lowering=False, debug=False,
        enable_asserts=False, num_devices=1 if single else NCORES,
    )
    # ---------------- per-core external I/O ----------------
    ut_d = nc.dram_tensor("ut", [P, 2, LC], BF16, kind="ExternalInput").ap()
    csn_d = nc.dram_tensor("csn", [P, NCH, 2, T], BF16, kind="ExternalInput").ap()
    tt_d = nc.dram_tensor("tt", [P, NCH, 2, T], BF16, kind="ExternalInput").ap()
    wb_d = nc.dram_tensor("wb", [P, _WB_LEN], BF16, kind="ExternalInput").ap()
    wc_d = nc.dram_tensor("wc", [P, _WC_LEN], BF16, kind="ExternalInput").ap()
    fb_d = nc.dram_tensor("fb", [P, T + 2 * NCORES], F32, kind="ExternalInput").ap()
    # natural (p, a, h) order to match the SBUF tile; host re-lays out
    out_d = nc.dram_tensor("out", [NCH, P, 4, H], BF16, kind="ExternalOutput").ap()

    with tile.TileContext(nc) as tc:
        _body(tc, nc, ut_d, csn_d, tt_d, wb_d, wc_d, fb_d, out_d, single=single)
    nc.compile()
    return nc


def _body(tc, nc, ut_d, csn_d, tt_d, wb_d, wc_d, fb_d, out_d, single=False):
    with (
        tc.tile_pool(name="const", bufs=1) as cpool,
        tc.tile_pool(name="span", bufs=1) as spool,
        tc.tile_pool(name="work", bufs=3) as wpool,
        tc.tile_pool(name="pbu", bufs=1, space="PSUM") as ppool,
        tc.tile_pool(name="pout", bufs=3, space="PSUM") as opool,
        tc.tile_pool(name="dram", bufs=1, space="DRAM") as dpool,
    ):
        # ---- constants; issue order = dependency order for chunk 0 ----
        wb_sb = cpool.tile([P, _WB_LEN], BF16)
        wc_sb = cpool.tile([P, _WC_LEN], BF16)
        fb_sb = cpool.tile([P, T + 2 * NCORES], F32)
        csn_sb = cpool.tile([P, NCH, 2, T], BF16)
        tt_sb = cpool.tile([P, NCH, 2, T], BF16)
        nc.sync.dma_start(wb_sb[:], wb_d)
        rrep = fb_sb[:, 0:T]
        wgr = fb_sb[:, T:T + NCORES]
        wgi = fb_sb[:, T + NCORES:T + 2 * NCORES]
        crt = wc_sb[:, _WC_CRT:_WC_CRT + H]
        cit = wc_sb[:, _WC_CIT:_WC_CIT + H]

        # ---- span-persistent state ----
        yr_sb = spool.tile([P, LC], BF16)         # scan outputs (rotated basis)
        yi_sb = spool.tile([P, LC], BF16)
        xr_sb = spool.tile([P, LC], BF16)         # unrotated state
        xi_sb = spool.tile([P, LC], BF16)
        ob1_sb = spool.tile([P, NCH, 4, H], F32)  # carry-free partial output

        # =============== phase 1: Bu, rotate, scan ===============
        for c in range(NCH):
            t0 = c * T
            utc = wpool.tile([P, 2, T], BF16, tag="utc")
            nc.sync.dma_start(utc[:], ut_d[:, :, t0:t0 + T])
            nc.scalar.dma_start(csn_sb[:, c], csn_d[:, c])
            if c == 0:
                nc.scalar.dma_start(fb_sb[:], fb_d)
            if c == 1:
                nc.sync.dma_start(wc_sb[:], wc_d)
            pbr = ppool.tile([P, T], F32, tag="pbr")
            pbi = ppool.tile([P, T], F32, tag="pbi")
            nc.tensor.matmul(pbr[:], wb_sb[:, _WB_BTR0:_WB_BTR0 + P],
                             utc[:, 0, :], start=True, stop=False)
            nc.tensor.matmul(pbr[:], wb_sb[:, _WB_BTR1:_WB_BTR1 + P],
                             utc[:, 1, :], start=False, stop=True)
            nc.tensor.matmul(pbi[:], wb_sb[:, _WB_BTI0:_WB_BTI0 + P],
                             utc[:, 0, :], start=True, stop=False)
            nc.tensor.matmul(pbi[:], wb_sb[:, _WB_BTI1:_WB_BTI1 + P],
                             utc[:, 1, :], start=False, stop=True)
            cs = csn_sb[:, c, 0, :]
            sn = csn_sb[:, c, 1, :]
            # PSUM -> SBUF bf16 on Act (gpsimd cannot touch PSUM; bf16
            # doubles DVE rate downstream)
            bur = wpool.tile([P, T], BF16, tag="bur")
            bui = wpool.tile([P, T], BF16, tag="bui")
            nc.scalar.copy(bur[:], pbr[:])
            nc.scalar.copy(bui[:], pbi[:])
            # w = e^{-i th t} * Bu
            m1 = wpool.tile([P, T], BF16, tag="m1")
            m2 = wpool.tile([P, T], BF16, tag="m2")
            m3 = wpool.tile([P, T], BF16, tag="m3")
            m4 = wpool.tile([P, T], BF16, tag="m4")
            wr = wpool.tile([P, T], BF16, tag="wr")
            wi = wpool.tile([P, T], BF16, tag="wi")
            nc.vector.tensor_mul(m1[:], cs, bur[:])
            nc.gpsimd.tensor_mul(m4[:], sn, bur[:])
            nc.vector.tensor_mul(m2[:], sn, bui[:])
            nc.vector.tensor_add(wr[:], m1[:], m2[:])
            nc.vector.tensor_mul(m3[:], cs, bui[:])
            nc.gpsimd.tensor_sub(wi[:], m3[:], m4[:])
            # phase-2-only table streams in behind the phase-1 deps
            nc.scalar.dma_start(tt_sb[:, c], tt_d[:, c])
            # chained scans (span-global basis: init = prev chunk's last col)
            init_r = 0.0 if c == 0 else yr_sb[:, t0 - 1:t0]
            init_i = 0.0 if c == 0 else yi_sb[:, t0 - 1:t0]
            nc.vector.tensor_tensor_scan(
                yr_sb[:, t0:t0 + T], rrep, wr[:], init_r, op0=MUL, op1=ADD)
            nc.vector.tensor_tensor_scan(
                yi_sb[:, t0:t0 + T], rrep, wi[:], init_i, op0=MUL, op1=ADD)

        # =============== kick off cross-core carry exchange ===============
        e_loc = dpool.tile([P, 2], F32)
        e_all = dpool.tile([NCORES, P, 2], F32)
        e_sb = cpool.tile([P, 2], F32)
        with tc.high_priority():
            nc.vector.tensor_copy(e_sb[:, 0:1], yr_sb[:, LC - 1:LC])
            nc.vector.tensor_copy(e_sb[:, 1:2], yi_sb[:, LC - 1:LC])
            nc.sync.dma_start(e_loc[:], e_sb[:])
            if single:
                nc.scalar.dma_start(e_all[0], e_loc[:])
            else:
                nc.gpsimd.collective_compute(
                    "AllGather", mybir.AluOpType.bypass,
                    replica_groups=[list(range(NCORES))],
                    ins=[e_loc.opt()], outs=[e_all.opt()])

        def p2a(c):
            """unrotate x = e^{+i th t} * y (collective-independent)"""
            t0 = c * T
            cs = csn_sb[:, c, 0, :]
            sn = csn_sb[:, c, 1, :]
            n1 = wpool.tile([P, T], BF16, tag="n1")
            n2 = wpool.tile([P, T], BF16, tag="n2")
            n3 = wpool.tile([P, T], BF16, tag="n3")
            n4 = wpool.tile([P, T], BF16, tag="n4")
            eng = nc.gpsimd if c >= POOL_P2A_FROM else nc.vector
            eng.tensor_mul(n2[:], sn, yi_sb[:, t0:t0 + T])
            nc.vector.tensor_mul(n1[:], cs, yr_sb[:, t0:t0 + T])
            nc.vector.tensor_sub(xr_sb[:, t0:t0 + T], n1[:], n2[:])
            eng.tensor_mul(n4[:], sn, yr_sb[:, t0:t0 + T])
            nc.vector.tensor_mul(n3[:], cs, yi_sb[:, t0:t0 + T])
            nc.vector.tensor_add(xi_sb[:, t0:t0 + T], n3[:], n4[:])

        def p2p(c):
            """carry-free partial projection 2Re(C x) -> ob1 (f32, exact);
            runs pre-W1 on otherwise-idle PE + Act"""
            t0 = c * T
            po = opool.tile([P, 4, H], F32, tag="po")
            for a in range(4):
                sl = slice(t0 + a * P, t0 + (a + 1) * P)
                nc.tensor.matmul(po[:, a, :], xr_sb[:, sl], crt,
                                 start=True, stop=False)
                nc.tensor.matmul(po[:, a, :], xi_sb[:, sl], cit,
                                 start=False, stop=True)
            nc.scalar.copy(ob1_sb[:, c], po[:])

        def p2c(c):
            """carry correction (TR W1 + TI W2) + final combine + store"""
            po = opool.tile([P, 4, H], F32, tag="po")
            ob = wpool.tile([P, 4, H], BF16, tag="ob")
            for a in range(4):
                nc.tensor.matmul(po[:, a, :], tt_sb[:, c, 0, a * P:(a + 1) * P],
                                 w1[:], start=True, stop=False)
                nc.tensor.matmul(po[:, a, :], tt_sb[:, c, 1, a * P:(a + 1) * P],
                                 w2[:], start=False, stop=True)
            # DVE is drained of unrot work by now; PSUM read keeps Act free
            nc.vector.tensor_add(ob[:], po[:], ob1_sb[:, c])
            (nc.sync if c % 2 == 0 else nc.scalar).dma_start(out_d[c], ob[:])

        # unrot + partial projection lead; the carry chain hides behind them
        p2a(0)
        p2p(0)
        p2a(1)
        p2p(1)
        p2a(2)
        p2p(2)

        # =============== carry combine -> W1/W2 ===============
        # high_priority: W1/W2 gate ~16us of PE projection work; don't let
        # the tile scheduler push these behind the remaining unrot chunks
        eall_sb = cpool.tile([P, NCORES, 2], F32)
        ta = cpool.tile([P, NCORES], F32)
        tb = cpool.tile([P, NCORES], F32)
        cr = cpool.tile([P, 1], F32)
        ci = cpool.tile([P, 1], F32)
        w1 = cpool.tile([P, H], BF16)
        w2 = cpool.tile([P, H], BF16)
        tw1 = cpool.tile([P, H], BF16)
        tw2 = cpool.tile([P, H], BF16)
        with tc.high_priority():
            nc.sync.dma_start(eall_sb[:],
                              e_all[:, :, :].rearrange("j p c -> p j c"))
            er_v = eall_sb[:, :, 0]
            ei_v = eall_sb[:, :, 1]
            nc.vector.tensor_mul(ta[:], wgr, er_v)
            nc.vector.tensor_mul(tb[:], wgi, ei_v)
            nc.vector.tensor_sub(ta[:], ta[:], tb[:])
            nc.vector.tensor_reduce(cr[:], ta[:], axis=AX, op=ADD)
            nc.vector.tensor_mul(ta[:], wgi, er_v)
            nc.vector.tensor_mul(tb[:], wgr, ei_v)
            nc.vector.tensor_add(ta[:], ta[:], tb[:])
            nc.vector.tensor_reduce(ci[:], ta[:], axis=AX, op=ADD)
            nc.vector.tensor_scalar_mul(tw1[:], cit, ci[:])
            nc.vector.scalar_tensor_tensor(w1[:], crt, cr[:], tw1[:],
                                           op0=MUL, op1=ADD)
            nc.vector.tensor_scalar_mul(tw2[:], crt, ci[:])
            nc.vector.scalar_tensor_tensor(w2[:], cit, cr[:], tw2[:],
                                           op0=MUL, op1=mybir.AluOpType.subtract)

        # =============== remaining unrot+partials, then corrections ========
        for c in range(3, NCH):
            p2a(c)
            p2p(c)
        for c in range(NCH):
            p2c(c)


_NC_CACHE = {}


def _get_nc():
    if "nc" not in _NC_CACHE:
        _NC_CACHE["nc"] = _build_kernel()
    return _NC_CACHE["nc"]


def _host_prep(Lambda_re, Lambda_im, B, C, D, log_step, input_sequence):
    """f64 host-side parameter/table preparation -> per-core input maps."""
    Lam = Lambda_re.astype(np.float64) + 1j * Lambda_im.astype(np.float64)
    step = np.exp(log_step.astype(np.float64))
    A = np.exp(Lam * step)                        # (P,)
    r = np.abs(A)
    th = np.imag(Lam * step)
    Bt = B[..., 0].astype(np.float64) + 1j * B[..., 1].astype(np.float64)
    Bbar = ((A - 1.0) / Lam)[:, None] * Bt        # (P, H)
    Ct = C[..., 0].astype(np.float64) + 1j * C[..., 1].astype(np.float64)  # (H, P)

    s = np.arange(LC, dtype=np.float64)
    ang = th[:, None] * s[None, :]
    cs = np.cos(ang)
    sn = np.sin(ang)
    csn = np.stack([cs.reshape(P, NCH, T), sn.reshape(P, NCH, T)],
                   axis=2).astype(NPBF16)          # [P, NCH, 2, T]
    # TR/TI = r^{t+1} (cos, sin)(th t): the e^{i th (t+1)} of A^{t+1} is
    # split as e^{i th t} here x e^{i th} inside the host carry weights
    rp = np.exp(np.log(r)[:, None] * (s[None, :] + 1.0))
    tr = rp * cs
    ti = rp * sn
    tt = np.stack([tr.reshape(P, NCH, T), ti.reshape(P, NCH, T)],
                  axis=2).astype(NPBF16)           # [P, NCH, 2, T]

    Br = np.real(Bbar).T                          # (256h, 128p)
    Bi = np.imag(Bbar).T
    crt = 2.0 * np.real(Ct).T                     # (128p, 256h)
    cit = -2.0 * np.imag(Ct).T
    wb = np.concatenate([Br[0:P], Br[P:H], Bi[0:P], Bi[P:H]],
                        axis=1).astype(NPBF16)
    wc = np.concatenate([crt, cit], axis=1).astype(NPBF16)

    rrep = np.broadcast_to(r[:, None], (P, T)).astype(np.float32)
    ALC = A ** LC
    eE = np.exp(1j * th * LC)      # includes the carry-in e^{i th} rotation
    # W'[i, j] = ALC^{i-1-j} * eE  for j < i
    wgc = np.zeros((NCORES, P, NCORES), np.complex128)
    pw = np.ones((P,), np.complex128)
    for k in range(NCORES - 1):
        w = pw * eE
        for j in range(NCORES - 1 - k):
            wgc[j + k + 1, :, j] = w
        pw = pw * ALC

    ub = input_sequence.astype(NPBF16)
    uT = ub.T                                     # (256, L) view

    in_maps = []
    for i in range(NCORES):
        utc = np.ascontiguousarray(
            uT[:, i * LC:(i + 1) * LC].reshape(2, P, LC).transpose(1, 0, 2))
        fb = np.concatenate(
            [rrep,
             np.ascontiguousarray(np.real(wgc[i])).astype(np.float32),
             np.ascontiguousarray(np.imag(wgc[i])).astype(np.float32)],
            axis=1)
        in_maps.append({
            "ut": utc,
            "csn": csn,
            "tt": tt,
            "wb": wb,
            "wc": wc,
            "fb": fb,
        })
    return in_maps


def kernel(Lambda_re, Lambda_im, B, C, D, log_step, input_sequence):
    in_maps = _host_prep(Lambda_re, Lambda_im, B, C, D, log_step,
                         input_sequence)
    nc = _get_nc()
    res = run_bass_kernel_spmd(nc, in_maps, list(range(NCORES)))
    out = np.concatenate(
        [_unscramble(res.results[i]["out"]) for i in range(NCORES)], axis=0)
    # D*u is cheaper on the host than 8 PE matmuls per chunk on device
    out += D.astype(np.float32) * input_sequence
    return out


def _unscramble(out_arr):
    """device layout [NCH, P, 4, H] (p-major) bf16 -> time-major [LC, H] f32"""
    return (np.asarray(out_arr).astype(np.float32)
            .transpose(0, 2, 1, 3).reshape(LC, H))


if __name__ == "__main__":
    pass
